# revision 3
# baseline (speedup 1.0000x reference)
"""ExLlama q4 dequant + matmul (tensor-parallel over out_features) on 8 trn2 cores.

Math (per core, N_loc = 28672/8 = 3584 columns):
  out[t,n] = sum_k x[t,k] * s[g(k),n] * (q[k,n] - (z[g(k),n]+1)) + bias[n]
           = sum_k x[t,k]*s[g,n]*q[k,n]  -  sum_g A[t,g]*(z+1)[g,n]*s[g,n] + bias[n]
  with A[t,g] = sum_{k in g} x[t,k] (host-computed, tiny).

Device pipeline per core ("Design W2" — x stationary, dequantized weights moving,
big-chunk DMA):
  - qweight host-permuted into 16 container tiles [128, N_loc] u16 (partition p
    holds k of group p//2; nibble c of word (jt,p,n) is k = (p//2)*128 +
    (jt*2+p%2)*4 + c), then packed per-partition-contiguous as
    [128, NJT*N_loc] and DMA'd in NDMA big chunks (~3.7 MB each).
  - All constants (sexp | xt | z65 | r65) packed into ONE [128, 9248] fp16
    tensor, one DMA.
  - DVE extract (4x): (u16 & (0xF<<4c)) -> u16 = q*16^c; DVE scale (2x):
    tensor_tensor mult with sexp -> w~ fp16.
  - PE: stationary = xt slice [128,32] (x*16^-c permuted), moving = w~ in 7
    chunks of N=512; PSUM [32, 3584] accumulates over all 64 passes + fixup
    matmul (lhsT=r65=[-A.T;1], rhs=z65=[(z+1)*s;bias]).
  - One ScalarE drain PSUM->SBUF fp16, one out DMA [32, 3584].
"""

import numpy as np

GROUP_SIZE = 128
IN_FEATURES = 8192
OUT_FEATURES = 28672
TOKENS = 32
N_CORES = 8
N_LOC = OUT_FEATURES // N_CORES          # 3584
NJT = IN_FEATURES // (GROUP_SIZE * 4)    # 16 container tiles
G = IN_FEATURES // GROUP_SIZE            # 64 groups
MASKS = (0x000F, 0x00F0, 0x0F00, 0xF000)
MMCH = 512                               # moving cols per matmul (1 PSUM bank)
NDMA = 4                                 # wq DMA chunks (NJT % NDMA == 0)
JPC = NJT // NDMA                        # jt tiles per chunk

# packed const layout (columns, fp16)
C_SEXP = 0
C_XT = N_LOC                             # 3584
C_Z65 = C_XT + NJT * 4 * TOKENS          # 5632
C_R65 = C_Z65 + N_LOC                    # 9216
C_W = C_R65 + TOKENS                     # 9248

_PROGRAM_CACHE = {}


# ---------------------------------------------------------------- host prep

def _k_index_map():
    """k(jt, p, c) = (p//2)*128 + (jt*2 + p%2)*4 + c  -> [NJT, 128, 4] int."""
    jt = np.arange(NJT)[:, None, None]
    p = np.arange(128)[None, :, None]
    c = np.arange(4)[None, None, :]
    return (p // 2) * GROUP_SIZE + (jt * 2 + (p % 2)) * 4 + c


def _prep_wq(qw_slice):
    """[1024, N_loc] int32 -> [128, NJT*N_loc] uint16 packed container tiles."""
    nloc = qw_slice.shape[1]
    qb = np.ascontiguousarray(qw_slice).view(np.uint8).reshape(1024, nloc, 4)
    # byte kp = 4*kk + b holds nibbles for k = 2kp (lo), 2kp+1 (hi)
    qb_kp = np.ascontiguousarray(qb.transpose(0, 2, 1)).reshape(4096, nloc)
    jt = np.arange(NJT)[:, None]
    p = np.arange(128)[None, :]
    kp0 = (p // 2) * 64 + (jt * 2 + (p % 2)) * 2      # [NJT, 128]
    b2 = np.stack([qb_kp[kp0], qb_kp[kp0 + 1]], axis=-1)  # [NJT,128,nloc,2] u8
    wq = np.ascontiguousarray(b2).view(np.uint16)[..., 0]  # [NJT,128,nloc]
    return np.ascontiguousarray(wq.transpose(1, 0, 2)).reshape(128, NJT * nloc)


def _prep_const(x, qz_slice, s_slice, b_slice):
    """Pack [sexp | xt | z65 | r65] into one [128, C_W] fp16 array."""
    nloc = s_slice.shape[1]
    cst = np.zeros((128, C_W), dtype=np.float16)
    # sexp
    cst[:, C_SEXP:C_SEXP + nloc] = np.repeat(
        s_slice.astype(np.float16), 2, axis=0)
    # xt
    kmap = _k_index_map()
    xf = x.astype(np.float32)
    for jt in range(NJT):
        for c in range(4):
            blk = xf[:, kmap[jt, :, c]].T * (2.0 ** (-4 * c))   # [128, 32]
            col = C_XT + (jt * 4 + c) * TOKENS
            cst[:, col:col + TOKENS] = blk.astype(np.float16)
    # z65 rows 0..63 = (z+1)*s, row 64 = bias
    shifts = (np.arange(8, dtype=np.uint32) * 4)[None, None, :]
    z = ((qz_slice.astype(np.uint32)[:, :, None] >> shifts) & 15)
    z = z.reshape(G, nloc).astype(np.float32)
    cst[:G, C_Z65:C_Z65 + nloc] = ((z + 1.0) * s_slice.astype(np.float32)
                                   ).astype(np.float16)
    cst[G, C_Z65:C_Z65 + nloc] = b_slice
    # r65 rows 0..63 = -A.T, row 64 = ones
    A = x.astype(np.float32).reshape(TOKENS, G, GROUP_SIZE).sum(axis=2)
    cst[:G, C_R65:C_R65 + TOKENS] = (-A.T).astype(np.float16)
    cst[G, C_R65:C_R65 + TOKENS] = 1.0
    return cst


# ---------------------------------------------------------------- device program

GP_EVERY = 0     # move every Nth scale-mult to GpSimd (0 = off)


def _build_program(nloc, loop_r=1, gp_every=GP_EVERY):
    import concourse.bacc as bacc
    import concourse.mybir as mybir
    import concourse.tile as tile
    from concourse.alu_op_type import AluOpType

    dt = mybir.dt
    nch = nloc // MMCH

    nc = bacc.Bacc("TRN2", target_bir_lowering=False, debug=False,
                   num_devices=N_CORES)

    wq_d = nc.dram_tensor("wq", [128, NJT * nloc], dt.uint16,
                          kind="ExternalInput")
    cst_d = nc.dram_tensor("cst", [128, C_W], dt.float16,
                           kind="ExternalInput")
    out_d = nc.dram_tensor("out", [TOKENS, nloc], dt.float16,
                           kind="ExternalOutput")

    with tile.TileContext(nc) as tc:
        with (
            tc.tile_pool(name="const", bufs=1) as const_pool,
            tc.tile_pool(name="wq", bufs=2) as wq_pool,
            tc.tile_pool(name="ext", bufs=4) as ext_pool,
            tc.tile_pool(name="sw", bufs=6) as sw_pool,
            tc.tile_pool(name="psum", bufs=1, space="PSUM") as psum_pool,
        ):
            def emit_body():
                cst = const_pool.tile([128, C_W], dt.float16, tag="cst")
                nc.sync.dma_start(cst[:], cst_d[:])
                sexp = cst[:, C_SEXP:C_SEXP + nloc]

                psum = psum_pool.tile([TOKENS, nch * MMCH], dt.float32,
                                      tag="acc")

                for dc in range(NDMA):
                    wq_t = wq_pool.tile([128, JPC * nloc], dt.uint16)
                    nc.sync.dma_start(
                        wq_t[:], wq_d[:, dc * JPC * nloc:(dc + 1) * JPC * nloc])
                    for j in range(JPC):
                        jt = dc * JPC + j
                        for c in range(4):
                            ext = ext_pool.tile([128, nloc], dt.uint16)
                            nc.vector.tensor_scalar(
                                ext[:], wq_t[:, j * nloc:(j + 1) * nloc],
                                MASKS[c], None, AluOpType.bitwise_and)
                            sw = sw_pool.tile([128, nloc], dt.float16)
                            ip = jt * 4 + c
                            eng = (nc.gpsimd if gp_every and
                                   ip % gp_every == gp_every - 1
                                   else nc.vector)
                            eng.tensor_tensor(
                                sw[:], ext[:], sexp, AluOpType.mult)
                            xcol = C_XT + ip * TOKENS
                            for ci in range(nch):
                                nc.tensor.matmul(
                                    psum[:, ci * MMCH:(ci + 1) * MMCH],
                                    cst[:, xcol:xcol + TOKENS],
                                    sw[:, ci * MMCH:(ci + 1) * MMCH],
                                    start=(jt == 0 and c == 0),
                                    stop=False)

                for ci in range(nch):
                    nc.tensor.matmul(
                        psum[:, ci * MMCH:(ci + 1) * MMCH],
                        cst[0:G + 1, C_R65:C_R65 + TOKENS],
                        cst[0:G + 1, C_Z65 + ci * MMCH:C_Z65 + (ci + 1) * MMCH],
                        start=False,
                        stop=True)

                stg = const_pool.tile([TOKENS, nch * MMCH], dt.float16,
                                      tag="stg")
                nc.scalar.copy(stg[:], psum[:])
                nc.sync.dma_start(out_d[:], stg[:])

            if loop_r == 1:
                emit_body()
            else:
                with tc.For_i(0, loop_r, 1):
                    emit_body()

    nc.compile()
    return nc


def _get_program(nloc=N_LOC):
    if nloc not in _PROGRAM_CACHE:
        _PROGRAM_CACHE[nloc] = _build_program(nloc)
    return _PROGRAM_CACHE[nloc]


# ---------------------------------------------------------------- entry point

def make_in_maps(x, qweight, qzeros, scales, bias, nloc=N_LOC, n_cores=N_CORES):
    x = np.asarray(x)
    qweight = np.asarray(qweight)
    qzeros = np.asarray(qzeros)
    scales = np.asarray(scales)
    bias = np.asarray(bias)

    in_maps = []
    for core in range(n_cores):
        n0, n1 = core * nloc, (core + 1) * nloc
        s_slice = np.ascontiguousarray(scales[:, n0:n1]).astype(np.float16)
        qz_slice = np.ascontiguousarray(qzeros[:, n0 // 8:n1 // 8]).view(
            np.uint32)
        b_slice = np.ascontiguousarray(bias[n0:n1]).astype(np.float16)
        in_maps.append({
            "wq": _prep_wq(qweight[:, n0:n1]),
            "cst": _prep_const(x, qz_slice, s_slice, b_slice),
        })
    return in_maps


def assemble_output(results, nloc=N_LOC, n_cores=N_CORES):
    parts = [np.asarray(results[core]["out"]) for core in range(n_cores)]
    return np.ascontiguousarray(np.concatenate(parts, axis=1))


def kernel(x, qweight, qzeros, scales, bias):
    from concourse.bass_utils import run_bass_kernel_spmd

    nc = _get_program()
    in_maps = make_in_maps(x, qweight, qzeros, scales, bias)
    res = run_bass_kernel_spmd(nc, in_maps, list(range(N_CORES)))
    return assemble_output(res.results)
